# revision 36
# baseline (speedup 1.0000x reference)
"""ByteEncoder Trainium2 kernel (v3: bf16 GEMMs, fast collective path, tuned
engine assignment).

Model: h = embed[x]; y = Conv1d(k=4, s=4)(h); y = LN(y)*g+b; xb = y@bW.T+bb;
       h_t = lam*h_{t-1} + (1-lam)*xb_t (LRU scan); out = h@cW.T+cb.

Strategy (8 NeuronCores, data-parallel over (batch, half-sequence)):
  * embed+conv fused into LUT_j[v,o] = sum_d embed[v,d]*conv_w[o,d,j] (GEMM A),
    conv applied as y^T = sum_j LUT_j^T @ onehot_j (GEMM B). Channel-major
    [d, t] layout everywhere so the LRU scan maps to DVE tensor_tensor_scan.
  * All matmul operands are bf16 (weights pre-cast/folded on host; b2=bW@ln_b
    precomputed host-side as O(D^2) weight prep): bf16 stationary operands get
    the compiler-automatic fast-weight-load path; accumulation stays fp32.
  * Pipeline: A (DMA-paced) -> B + stats (t-chunk-outer, squares on Pool) ->
    normalize (sub Pool / mul DVE) -> D with single-ACT-op drain into u ->
    scans interleaved on DVE -> F on UNCORRECTED h while the pair AllGather of
    final states runs concurrently -> patch first 128 tokens (lam^129 ~ 1e-13)
    via a 16-matmul redo of the tt=0 tile.
  * Final-state exchange uses PE transposes + contiguous f32 DMAs (a strided
    2-byte scatter DMA measured 20us; the transpose path is ~1us) and an f32
    AllGather (bf16 collectives hit a slow CC path).
  * DMA split across SP (weights, outputs) + ACT (x, params) queues with the
    2MB x broadcast LAST so it cannot starve the phase-A weight stream.
"""

import sys

sys.path.insert(0, "/opt/trn_rl_repo")

from contextlib import ExitStack

import numpy as np
import ml_dtypes

import concourse.bass as bass
import concourse.tile as tile
from concourse import mybir

B, T, D = 4, 8192, 1024
NCORES = 8
TC = T // 4            # 2048 conv tokens per batch
TPC = TC // 2          # 1024 conv tokens per core
XPC = TPC * 4          # 4096 input tokens per core
V = 256                # vocab
P = 128
DT = D // P            # 8 d-tiles (also o-tiles)
VT = V // P            # 2 v-tiles
NJ = 4                 # conv taps
NT = TPC // 512        # 2 t-chunks of 512
NO = D // 512          # 2 o-chunks of 512

F32 = mybir.dt.float32
BF16 = mybir.dt.bfloat16
I32 = mybir.dt.int32
AF = mybir.ActivationFunctionType
OP = mybir.AluOpType

LN_EPS = 1e-5
NPBF = ml_dtypes.bfloat16


def _vec_view(dram_ap):
    """[D] dram vector -> [128, 8] view (partition p, free dt; d = dt*128+p)."""
    return dram_ap.rearrange("(dt p) -> p dt", p=P)


def build_nc():
    nc = bass.Bass(trn_type="TRN2", num_devices=NCORES)

    x_f = nc.declare_dram_parameter("x_f", [XPC], BF16, isOutput=False)
    etT = nc.declare_dram_parameter("etT", [P, DT, V], BF16, isOutput=False)
    cwT = nc.declare_dram_parameter("cwT", [NJ, 4, P, 2 * D], BF16, isOutput=False)
    bwtT = nc.declare_dram_parameter("bwtT", [P, DT, D], BF16, isOutput=False)
    cwtT = nc.declare_dram_parameter("cwtT", [P, DT, D], BF16, isOutput=False)
    conv_b = nc.declare_dram_parameter("conv_b", [D], F32, isOutput=False)
    log_lambda = nc.declare_dram_parameter("log_lambda", [D], F32, isOutput=False)
    b2v = nc.declare_dram_parameter("b2v", [D], F32, isOutput=False)
    bb = nc.declare_dram_parameter("bb", [D], F32, isOutput=False)
    cb = nc.declare_dram_parameter("cb", [D], F32, isOutput=False)
    parity = nc.declare_dram_parameter("parity", [1], F32, isOutput=False)
    out = nc.declare_dram_parameter("out", [TPC, D], F32, isOutput=True)

    fin_dram = nc.dram_tensor("fin_dram", [D], F32)
    fin_all = nc.dram_tensor("fin_all", [2, D], F32)

    with tile.TileContext(nc) as tc, ExitStack() as ctx, \
            nc.allow_low_precision(reason="bf16 matmul operands"):
        _body(ctx, tc, x_f.ap(), etT.ap(), cwT.ap(), bwtT.ap(), cwtT.ap(),
              conv_b.ap(), log_lambda.ap(), b2v.ap(), bb.ap(), cb.ap(),
              parity.ap(), out.ap(), fin_dram.ap(), fin_all.ap())
    _split_excess_waits(nc)
    return nc


def _split_excess_waits(nc, max_waits=1):
    """walrus codegen allows only one sync-wait slot per TPB instruction;
    hoist excess waits onto single-wait NoOps inserted just before the
    instruction on the same engine queue (queue order makes this exact)."""
    cnt = 0
    for f in nc.m.functions:
        for b in f.blocks:
            insts = list(b.instructions)
            out_list = []
            for inst in insts:
                si = inst.sync_info
                waits = list(si.on_wait) if si is not None and si.on_wait else []
                if len(waits) > max_waits:
                    for w in waits[:-max_waits]:
                        nop = mybir.InstNoOp(
                            name=f"waitsplit_{cnt}",
                            sync_info=mybir.SyncInfo(on_wait=[w], on_update=[]),
                        )
                        nop.engine = inst.engine
                        nc.inst_map[nop.name] = nop
                        cnt += 1
                        out_list.append(nop)
                    inst.sync_info = mybir.SyncInfo(
                        on_wait=waits[-max_waits:],
                        on_update=list(si.on_update) if si.on_update else [])
                out_list.append(inst)
            b.instructions = out_list
    return cnt


def _body(ctx, tc, x_f, etT, cwT, bwtT, cwtT, conv_b, log_lambda, b2v, bb, cb,
          parity, out, fin_dram, fin_all):
    nc = tc.nc

    big = ctx.enter_context(tc.tile_pool(name="big", bufs=1))
    small = ctx.enter_context(tc.tile_pool(name="small", bufs=1))
    cwpool = ctx.enter_context(tc.tile_pool(name="cwpool", bufs=3))
    ysubpool = ctx.enter_context(tc.tile_pool(name="ysubpool", bufs=3))
    stpool = ctx.enter_context(tc.tile_pool(name="stpool", bufs=3))
    lppool = ctx.enter_context(tc.tile_pool(name="lppool", bufs=2))
    pp = ctx.enter_context(tc.tile_pool(name="pp", bufs=8, space="PSUM"))

    _uid = [0]

    def bank(tag, shape, dtype):
        _uid[0] += 1
        return big.tile(list(shape), dtype, tag=tag, name=f"{tag}_{_uid[0]}")

    def psum(name, shape=(P, 512)):
        return pp.tile(list(shape), F32, tag="mm", name=name)

    # ---------------- SP queue: weight DMAs, in priority order ----------------
    et_t = bank("b_et", (P, DT, V), BF16)
    nc.sync.dma_start(out=et_t, in_=etT)
    cws = []
    for j in range(NJ):
        for hf in range(4):
            t_ = cwpool.tile([P, 2 * D], BF16, tag="cw", name=f"cw_{j}_{hf}")
            # alternate slabs across the two HWDGE queues for aggregate BW
            eng = nc.sync if (j * 4 + hf) % 2 == 0 else nc.scalar
            eng.dma_start(out=t_, in_=cwT[j, hf])
            cws.append(t_)
    bwt_t = bank("b_bwt", (P, DT, D), BF16)
    nc.sync.dma_start(out=bwt_t, in_=bwtT)
    cwt_t = bank("b_cwt", (P, DT, D), BF16)
    nc.sync.dma_start(out=cwt_t, in_=cwtT)

    # ---------------- ACT queue: small params first, 2MB x broadcast last ----
    convb_t = small.tile([P, DT], F32, tag="convb")
    ll_t = small.tile([P, DT], F32, tag="ll")
    bb_t = small.tile([P, DT], F32, tag="bb")
    b2_t = small.tile([P, DT], F32, tag="b2")
    nc.scalar.dma_start(out=convb_t, in_=_vec_view(conv_b))
    nc.scalar.dma_start(out=ll_t, in_=_vec_view(log_lambda))
    nc.scalar.dma_start(out=bb_t, in_=_vec_view(bb))
    nc.scalar.dma_start(out=b2_t, in_=_vec_view(b2v))
    parity_sb = small.tile([P, 1], F32, tag="parity")
    nc.scalar.dma_start(out=parity_sb, in_=parity.partition_broadcast(P))
    cb_bc = small.tile([P, D], F32, tag="cb")
    nc.scalar.dma_start(out=cb_bc, in_=cb.partition_broadcast(P))
    xbc = bank("b_x_h", (P, XPC), BF16)
    nc.scalar.dma_start(out=xbc, in_=x_f.partition_broadcast(P))

    # lam = sigmoid(exp(log_lambda)); ln(lam); lam powers for the carry patch
    e_t = small.tile([P, DT], F32, tag="e")
    lam_t = small.tile([P, DT], F32, tag="lam")
    ll2_t = small.tile([P, DT], F32, tag="ll2")
    nc.scalar.activation(out=e_t, in_=ll_t, func=AF.Exp)
    nc.scalar.activation(out=lam_t, in_=e_t, func=AF.Sigmoid)
    nc.scalar.activation(out=ll2_t, in_=lam_t, func=AF.Ln)

    # ---------------- DVE queue: iotas, constants, onehot ----------------
    oml_t = small.tile([P, DT], F32, tag="oml")
    nc.vector.tensor_scalar(out=oml_t, in0=lam_t, scalar1=-1.0, scalar2=1.0,
                            op0=OP.mult, op1=OP.add)
    lam16 = small.tile([P, DT], BF16, tag="lam16")
    nc.vector.tensor_copy(out=lam16, in_=lam_t)
    c0b = small.tile([P, DT], F32, tag="c0b")
    nc.vector.tensor_add(out=c0b, in0=b2_t, in1=bb_t)
    nc.vector.tensor_mul(out=c0b, in0=c0b, in1=oml_t)

    iota_v = small.tile([P, 1], I32, tag="iota_v")
    nc.gpsimd.iota(iota_v, [[0, 1]], base=0, channel_multiplier=1)
    iota_vf = small.tile([P, 1], F32, tag="iota_vf")
    nc.vector.tensor_copy(out=iota_vf, in_=iota_v)
    iota_b2 = small.tile([P, 1], F32, tag="iota_b2")
    nc.vector.tensor_scalar(out=iota_b2, in0=iota_vf, scalar1=float(P),
                            scalar2=None, op0=OP.add)
    tpos_i = small.tile([P, P], I32, tag="tpos_i")
    nc.gpsimd.iota(tpos_i, [[1, P]], base=1, channel_multiplier=0)
    tpos_f = small.tile([P, P], F32, tag="tpos_f")
    nc.vector.tensor_copy(out=tpos_f, in_=tpos_i)
    irow_i = small.tile([P, P], I32, tag="irow_i")
    nc.gpsimd.iota(irow_i, [[1, P]], base=0, channel_multiplier=0)
    irow_f = small.tile([P, P], F32, tag="irow_f")
    nc.vector.tensor_copy(out=irow_f, in_=irow_i)
    ident = small.tile([P, P], F32, tag="ident")
    nc.vector.tensor_scalar(out=ident, in0=irow_f, scalar1=iota_vf,
                            scalar2=None, op0=OP.is_equal)
    ones16 = small.tile([P, P], BF16, tag="ones16")
    nc.vector.memset(ones16, 1.0)
    eps_sb = small.tile([P, 1], F32, tag="eps")
    nc.vector.memset(eps_sb, LN_EPS)

    # lam^{t+1} for the first 128 tokens (patch window), computed early
    lp8 = small.tile([P, DT, P], F32, tag="lp8")
    for dt_ in range(DT):
        nc.scalar.activation(out=lp8[:, dt_, :], in_=tpos_f, func=AF.Exp,
                             scale=ll2_t[:, dt_:dt_ + 1])

    # onehot: oh[:, k=j*2+vt, t] = (x[4t+j] == v) in bf16
    oh_t = bank("b_oh_u", (P, DT, TPC), BF16)
    xv4 = xbc.rearrange("p (q t j) -> p q t j", q=4, j=NJ)
    for j in range(NJ):
        for vt in range(VT):
            iv = iota_vf if vt == 0 else iota_b2
            k = j * VT + vt
            for q in range(4):
                nc.vector.tensor_scalar(
                    out=oh_t[:, k, q * 256:(q + 1) * 256],
                    in0=xv4[:, q, :, j], scalar1=iv, scalar2=None,
                    op0=OP.is_equal)

    # ---------------- phase A: LUT_j[v, o] = embed^T conv_w ----------------
    lut_t = bank("b_lut", (P, DT, D), BF16)
    for j in range(NJ):
        ps = [[psum(f"psA_{j}_{vt}_{oc}") for oc in range(NO)] for vt in range(VT)]
        for dt_ in range(DT):
            hf, dq = dt_ // 2, dt_ % 2
            for vt in range(VT):
                for oc in range(NO):
                    nc.tensor.matmul(
                        ps[vt][oc],
                        et_t[:, dt_, vt * P:(vt + 1) * P],
                        cws[j * 4 + hf][:, dq * D + oc * 512:dq * D + (oc + 1) * 512],
                        start=(dt_ == 0), stop=(dt_ == DT - 1))
        for vt in range(VT):
            for oc in range(NO):
                # last tap's copies on DVE so B's first chain isn't gated on
                # the ACT queue draining
                if j == NJ - 1:
                    nc.vector.tensor_copy(
                        out=lut_t[:, j * VT + vt, oc * 512:(oc + 1) * 512],
                        in_=ps[vt][oc])
                else:
                    nc.scalar.activation(
                        out=lut_t[:, j * VT + vt, oc * 512:(oc + 1) * 512],
                        in_=ps[vt][oc], func=AF.Copy)

    # ---------------- phase B: conv GEMM y^T[o, t]; squares on Pool ----------
    y_t = bank("b_y", (P, DT, TPC), BF16)
    y2_t = bank("b_y2", (P, DT, TPC), BF16)

    _sc = [0]

    def stats_chain(tc_, src_t, dst, scale):
        sl = slice(tc_ * 512, (tc_ + 1) * 512)
        _sc[0] += 1
        ps_s = psum(f"psS_{_sc[0]}")
        for ot in range(DT):
            nc.tensor.matmul(ps_s, ones16, src_t[:, ot, sl],
                             start=(ot == 0), stop=(ot == DT - 1))
        nc.scalar.activation(out=dst, in_=ps_s, func=AF.Copy, scale=scale)

    ypp_t = bank("b_ypp", (P, DT, TPC), BF16)

    def emit_b_chunk(tc_, ot):
        sl = slice(tc_ * 512, (tc_ + 1) * 512)
        psy = psum(f"psB_{tc_}_{ot}")
        for j in range(NJ):
            for vt in range(VT):
                nc.tensor.matmul(
                    psy,
                    lut_t[:, j * VT + vt, ot * P:(ot + 1) * P],
                    oh_t[:, j * VT + vt, sl],
                    start=(j == 0 and vt == 0),
                    stop=(j == NJ - 1 and vt == VT - 1))
        nc.scalar.activation(out=y_t[:, ot, sl], in_=psy, func=AF.Identity,
                             bias=convb_t[:, ot:ot + 1], scale=1.0)
        nc.gpsimd.tensor_mul(out=y2_t[:, ot, sl], in0=y_t[:, ot, sl],
                             in1=y_t[:, ot, sl])

    def emit_ln_chain(tc_, mub_c, e2_c):
        # var = E[y^2]-mu^2, r = 1/sqrt(var+eps), then normalize this chunk:
        # ypp = (y - mu) * r  (sub on Pool, mul on DVE, both [P, 512] tiles)
        sl = slice(tc_ * 512, (tc_ + 1) * 512)
        mu2 = stpool.tile([P, 512], F32, tag="mu2", name=f"mu2_{tc_}")
        nc.vector.tensor_mul(out=mu2, in0=mub_c, in1=mub_c)
        rb_c = stpool.tile([P, 512], F32, tag="rbp", name=f"rb_{tc_}")
        nc.vector.tensor_sub(out=rb_c, in0=e2_c, in1=mu2)
        nc.scalar.activation(out=rb_c, in_=rb_c, func=AF.Sqrt, bias=eps_sb)
        nc.vector.reciprocal(out=rb_c, in_=rb_c)
        for dt_ in range(DT):
            t_ = ysubpool.tile([P, 512], BF16, tag="ysub",
                               name=f"ysub_{tc_}_{dt_}")
            nc.gpsimd.tensor_sub(out=t_, in0=y_t[:, dt_, sl], in1=mub_c)
            nc.vector.tensor_mul(out=ypp_t[:, dt_, sl], in0=t_, in1=rb_c)

    # PE order: B-tc0, sum0, B-tc1[ot0], sumsq0, B-tc1[ot1..7], sum1, sumsq1
    mub0 = stpool.tile([P, 512], BF16, tag="mub", name="mub0")
    e20 = stpool.tile([P, 512], F32, tag="e2p", name="e20")
    mub1 = stpool.tile([P, 512], BF16, tag="mub", name="mub1")
    e21 = stpool.tile([P, 512], F32, tag="e2p", name="e21")
    for ot in range(DT):
        emit_b_chunk(0, ot)
    stats_chain(0, y_t, mub0, 1.0 / D)
    emit_b_chunk(1, 0)
    stats_chain(0, y2_t, e20, 1.0 / D)
    emit_ln_chain(0, mub0, e20)
    for ot in range(1, DT):
        emit_b_chunk(1, ot)
    stats_chain(1, y_t, mub1, 1.0 / D)
    stats_chain(1, y2_t, e21, 1.0 / D)
    emit_ln_chain(1, mub1, e21)

    # ------------- phase D: b-projection + scan, interleaved ----------------
    u_t = bank("b_oh_u", (P, DT, TPC), BF16)
    h_t = bank("b_x_h", (P, DT, TPC), BF16)
    fin32 = small.tile([P, DT], F32, tag="fin32")
    lam_bc = [lam16[:, k:k + 1].broadcast_to((P, TPC)) for k in range(DT)]

    def emit_scan(so):
        nc.vector.tensor_tensor_scan(
            out=h_t[:, so, :], data0=lam_bc[so], data1=u_t[:, so, :],
            initial=0.0, op0=OP.mult, op1=OP.add)
        nc.vector.tensor_copy(out=fin32[:, so:so + 1],
                              in_=h_t[:, so, TPC - 1:TPC])

    for tc_ in range(NT):
        sl = slice(tc_ * 512, (tc_ + 1) * 512)
        for ot in range(DT):
            psx = psum(f"psD_{ot}_{tc_}")
            for dt_ in range(DT):
                nc.tensor.matmul(
                    psx, bwt_t[:, dt_, ot * P:(ot + 1) * P],
                    ypp_t[:, dt_, sl],
                    start=(dt_ == 0), stop=(dt_ == DT - 1))
            nc.scalar.activation(out=u_t[:, ot, sl], in_=psx, func=AF.Identity,
                                 scale=oml_t[:, ot:ot + 1],
                                 bias=c0b[:, ot:ot + 1])
            if tc_ == 1 and ot >= 1:
                emit_scan(ot - 1)
    emit_scan(DT - 1)

    # ---------------- phase F (main): c-projection for tt = 1..7 -------------
    # The final-state exchange is interleaved: after tt=1's chains, PE
    # transposes fin32 -> contiguous f32 DMA -> pair AllGather, which
    # completes while the remaining tt tiles run.
    fin_sb = small.tile([DT, P], F32, tag="fin_sb")
    carrysb = small.tile([DT, P], F32, tag="carrysb")
    for tt in range(1, DT):
        stage = stpool.tile([P, D], F32, tag="stage", name=f"stage_{tt}")
        for oc in range(NO):
            pso = psum(f"psF_{tt}_{oc}")
            for dt_ in range(DT):
                nc.tensor.matmul(
                    pso, h_t[:, dt_, tt * P:(tt + 1) * P],
                    cwt_t[:, dt_, oc * 512:(oc + 1) * 512],
                    start=(dt_ == 0), stop=(dt_ == DT - 1))
            nc.vector.scalar_tensor_tensor(
                out=stage[:, oc * 512:(oc + 1) * 512],
                in0=cb_bc[:, oc * 512:(oc + 1) * 512], scalar=1.0, in1=pso,
                op0=OP.mult, op1=OP.add)
        nc.sync.dma_start(out=out[tt * P:(tt + 1) * P, :], in_=stage)
        if tt == 1:
            ps_ft = psum("ps_ftrans")
            nc.tensor.transpose(ps_ft[0:DT, 0:P], fin32, ident)
            nc.scalar.activation(out=fin_sb, in_=ps_ft[0:DT, 0:P], func=AF.Copy)
            nc.sync.dma_start(out=fin_dram.rearrange("(r c) -> r c", r=DT),
                              in_=fin_sb)
            nc.gpsimd.collective_compute(
                "AllGather", OP.bypass,
                replica_groups=[[0, 1], [2, 3], [4, 5], [6, 7]],
                ins=[fin_dram], outs=[fin_all])
            nc.scalar.dma_start(
                out=carrysb, in_=fin_all[0].rearrange("(r c) -> r c", r=DT))

    # transpose the carry back to [p, dt] once the AllGather lands
    ps_bt = psum("ps_btrans")
    nc.tensor.transpose(ps_bt[:, 0:DT], carrysb, ident[0:DT, 0:DT])
    carrym = small.tile([P, DT], F32, tag="carrym")
    nc.vector.tensor_scalar(out=carrym, in0=ps_bt[:, 0:DT], scalar1=parity_sb,
                            scalar2=None, op0=OP.mult)
    hc_t = small.tile([P, DT, P], BF16, tag="hc")
    for dt_ in range(DT):
        corr = lppool.tile([P, P], BF16, tag="corr", name=f"corr_{dt_}")
        nc.vector.tensor_scalar(out=corr, in0=lp8[:, dt_, :],
                                scalar1=carrym[:, dt_:dt_ + 1], scalar2=None,
                                op0=OP.mult)
        nc.gpsimd.tensor_add(out=hc_t[:, dt_, :], in0=h_t[:, dt_, 0:P],
                             in1=corr)
    stage0 = stpool.tile([P, D], F32, tag="stage", name="stage_tt0")
    for oc in range(NO):
        pso = psum(f"psF0_{oc}")
        for dt_ in range(DT):
            nc.tensor.matmul(pso, hc_t[:, dt_, :],
                             cwt_t[:, dt_, oc * 512:(oc + 1) * 512],
                             start=(dt_ == 0), stop=(dt_ == DT - 1))
        nc.vector.scalar_tensor_tensor(
            out=stage0[:, oc * 512:(oc + 1) * 512],
            in0=cb_bc[:, oc * 512:(oc + 1) * 512], scalar=1.0, in1=pso,
            op0=OP.mult, op1=OP.add)
    nc.sync.dma_start(out=out[0:P, :], in_=stage0)


_NC_CACHE = None


def _get_nc():
    global _NC_CACHE
    if _NC_CACHE is None:
        _NC_CACHE = build_nc()
    return _NC_CACHE


def _in_maps(x, embed, conv_w, conv_b, ln_g, ln_b, log_lambda, bW, bb, cW, cb):
    f = lambda a: np.ascontiguousarray(np.asarray(a, dtype=np.float32))
    bf = lambda a: np.ascontiguousarray(np.asarray(a, dtype=np.float32).astype(NPBF))
    x = np.asarray(x)
    # embed^T -> [p, dt, v]
    etT = bf(np.asarray(embed, np.float32).T.reshape(DT, P, V).transpose(1, 0, 2))
    # conv_w [o,d,k] -> [j, half, p, dq*o]
    cw = np.asarray(conv_w, np.float32).transpose(2, 1, 0)       # [k, d, o]
    cw = cw.reshape(NJ, DT, P, D).transpose(0, 2, 1, 3)          # [j, p, dt, o]
    cwT = bf(cw.reshape(NJ, P, 4, 2, D).transpose(0, 2, 1, 3, 4)
             .reshape(NJ, 4, P, 2 * D))
    # weight prep (O(D^2), init-time): fold ln gamma into bW; b2 = bW @ ln_b
    bW32 = np.asarray(bW, np.float32)
    bWg = bW32 * np.asarray(ln_g, np.float32)[None, :]
    b2 = bW32 @ np.asarray(ln_b, np.float32)
    bwtT = bf(bWg.T.reshape(DT, P, D).transpose(1, 0, 2))
    cwtT = bf(np.asarray(cW, np.float32).T.reshape(DT, P, D).transpose(1, 0, 2))
    shared = dict(etT=etT, cwT=cwT, bwtT=bwtT, cwtT=cwtT,
                  conv_b=f(conv_b), log_lambda=f(log_lambda),
                  b2v=f(b2), bb=f(bb), cb=f(cb))
    maps = []
    for c in range(NCORES):
        b, h = c // 2, c % 2
        xf = np.ascontiguousarray(
            x[b, h * XPC:(h + 1) * XPC].astype(np.float32).astype(NPBF))
        maps.append(dict(x_f=xf, parity=np.array([float(h)], np.float32),
                         **shared))
    return maps


def _unshard(results):
    out = np.empty((B, TC, D), np.float32)
    for c in range(NCORES):
        b, h = c // 2, c % 2
        out[b, h * TPC:(h + 1) * TPC, :] = results[c]["out"]
    return out


def run(trace=False, **inputs):
    from concourse.bass_utils import run_bass_kernel_spmd
    nc = _get_nc()
    maps = _in_maps(**inputs)
    res = run_bass_kernel_spmd(nc, maps, list(range(NCORES)), trace=trace)
    return _unshard(res.results), res


def kernel(**inputs):
    out, _ = run(trace=False, **inputs)
    return out


# revision 39
# speedup vs baseline: 1.0374x; 1.0374x over previous
"""ByteEncoder Trainium2 kernel (v3: bf16 GEMMs, fast collective path, tuned
engine assignment).

Model: h = embed[x]; y = Conv1d(k=4, s=4)(h); y = LN(y)*g+b; xb = y@bW.T+bb;
       h_t = lam*h_{t-1} + (1-lam)*xb_t (LRU scan); out = h@cW.T+cb.

Strategy (8 NeuronCores, data-parallel over (batch, half-sequence)):
  * embed+conv fused into LUT_j[v,o] = sum_d embed[v,d]*conv_w[o,d,j] (GEMM A),
    conv applied as y^T = sum_j LUT_j^T @ onehot_j (GEMM B). Channel-major
    [d, t] layout everywhere so the LRU scan maps to DVE tensor_tensor_scan.
  * All matmul operands are bf16 (weights pre-cast/folded on host; b2=bW@ln_b
    precomputed host-side as O(D^2) weight prep): bf16 stationary operands get
    the compiler-automatic fast-weight-load path; accumulation stays fp32.
  * Pipeline: A (DMA-paced) -> B + stats (t-chunk-outer, squares on Pool) ->
    normalize (sub Pool / mul DVE) -> D with single-ACT-op drain into u ->
    scans interleaved on DVE -> F on UNCORRECTED h while the pair AllGather of
    final states runs concurrently -> patch first 128 tokens (lam^129 ~ 1e-13)
    via a 16-matmul redo of the tt=0 tile.
  * Final-state exchange uses PE transposes + contiguous f32 DMAs (a strided
    2-byte scatter DMA measured 20us; the transpose path is ~1us) and an f32
    AllGather (bf16 collectives hit a slow CC path).
  * DMA split across SP (weights, outputs) + ACT (x, params) queues with the
    2MB x broadcast LAST so it cannot starve the phase-A weight stream.
"""

import sys

sys.path.insert(0, "/opt/trn_rl_repo")

from contextlib import ExitStack

import numpy as np
import ml_dtypes

import concourse.bass as bass
import concourse.tile as tile
from concourse import mybir

B, T, D = 4, 8192, 1024
NCORES = 8
TC = T // 4            # 2048 conv tokens per batch
TPC = TC // 2          # 1024 conv tokens per core
XPC = TPC * 4          # 4096 input tokens per core
V = 256                # vocab
P = 128
DT = D // P            # 8 d-tiles (also o-tiles)
VT = V // P            # 2 v-tiles
NJ = 4                 # conv taps
NT = TPC // 512        # 2 t-chunks of 512
NO = D // 512          # 2 o-chunks of 512

F32 = mybir.dt.float32
BF16 = mybir.dt.bfloat16
I32 = mybir.dt.int32
AF = mybir.ActivationFunctionType
OP = mybir.AluOpType

LN_EPS = 1e-5
NPBF = ml_dtypes.bfloat16


def _vec_view(dram_ap):
    """[D] dram vector -> [128, 8] view (partition p, free dt; d = dt*128+p)."""
    return dram_ap.rearrange("(dt p) -> p dt", p=P)


def build_nc():
    nc = bass.Bass(trn_type="TRN2", num_devices=NCORES)

    x_f = nc.declare_dram_parameter("x_f", [XPC], BF16, isOutput=False)
    etT = nc.declare_dram_parameter("etT", [P, DT, V], BF16, isOutput=False)
    cwT = nc.declare_dram_parameter("cwT", [NJ, 4, P, 2 * D], BF16, isOutput=False)
    bwtT = nc.declare_dram_parameter("bwtT", [P, DT, D], BF16, isOutput=False)
    cwtT = nc.declare_dram_parameter("cwtT", [P, DT, D], BF16, isOutput=False)
    conv_b = nc.declare_dram_parameter("conv_b", [D], F32, isOutput=False)
    log_lambda = nc.declare_dram_parameter("log_lambda", [D], F32, isOutput=False)
    b2v = nc.declare_dram_parameter("b2v", [D], F32, isOutput=False)
    bb = nc.declare_dram_parameter("bb", [D], F32, isOutput=False)
    cb = nc.declare_dram_parameter("cb", [D], F32, isOutput=False)
    parity = nc.declare_dram_parameter("parity", [1], F32, isOutput=False)
    out = nc.declare_dram_parameter("out", [TPC, D], F32, isOutput=True)

    fin_dram = nc.dram_tensor("fin_dram", [D], F32)
    fin_all = nc.dram_tensor("fin_all", [2, D], F32)

    with tile.TileContext(nc) as tc, ExitStack() as ctx, \
            nc.allow_low_precision(reason="bf16 matmul operands"):
        _body(ctx, tc, x_f.ap(), etT.ap(), cwT.ap(), bwtT.ap(), cwtT.ap(),
              conv_b.ap(), log_lambda.ap(), b2v.ap(), bb.ap(), cb.ap(),
              parity.ap(), out.ap(), fin_dram.ap(), fin_all.ap())
    _split_excess_waits(nc)
    return nc


def _split_excess_waits(nc, max_waits=1):
    """walrus codegen allows only one sync-wait slot per TPB instruction;
    hoist excess waits onto single-wait NoOps inserted just before the
    instruction on the same engine queue (queue order makes this exact)."""
    cnt = 0
    for f in nc.m.functions:
        for b in f.blocks:
            insts = list(b.instructions)
            out_list = []
            for inst in insts:
                si = inst.sync_info
                waits = list(si.on_wait) if si is not None and si.on_wait else []
                if len(waits) > max_waits:
                    for w in waits[:-max_waits]:
                        nop = mybir.InstNoOp(
                            name=f"waitsplit_{cnt}",
                            sync_info=mybir.SyncInfo(on_wait=[w], on_update=[]),
                        )
                        nop.engine = inst.engine
                        nc.inst_map[nop.name] = nop
                        cnt += 1
                        out_list.append(nop)
                    inst.sync_info = mybir.SyncInfo(
                        on_wait=waits[-max_waits:],
                        on_update=list(si.on_update) if si.on_update else [])
                out_list.append(inst)
            b.instructions = out_list
    return cnt


def _body(ctx, tc, x_f, etT, cwT, bwtT, cwtT, conv_b, log_lambda, b2v, bb, cb,
          parity, out, fin_dram, fin_all):
    nc = tc.nc

    big = ctx.enter_context(tc.tile_pool(name="big", bufs=1))
    small = ctx.enter_context(tc.tile_pool(name="small", bufs=1))
    cwpool = ctx.enter_context(tc.tile_pool(name="cwpool", bufs=3))
    ysubpool = ctx.enter_context(tc.tile_pool(name="ysubpool", bufs=3))
    stpool = ctx.enter_context(tc.tile_pool(name="stpool", bufs=3))
    lppool = ctx.enter_context(tc.tile_pool(name="lppool", bufs=2))
    pp = ctx.enter_context(tc.tile_pool(name="pp", bufs=8, space="PSUM"))

    _uid = [0]

    def bank(tag, shape, dtype):
        _uid[0] += 1
        return big.tile(list(shape), dtype, tag=tag, name=f"{tag}_{_uid[0]}")

    def psum(name, shape=(P, 512)):
        return pp.tile(list(shape), F32, tag="mm", name=name)

    # ---------------- SP queue: weight DMAs, in priority order ----------------
    et_t = bank("b_et", (P, DT, V), BF16)
    nc.sync.dma_start(out=et_t, in_=etT)
    cws = []
    for j in range(NJ):
        for hf in range(4):
            t_ = cwpool.tile([P, 2 * D], BF16, tag="cw", name=f"cw_{j}_{hf}")
            nc.sync.dma_start(out=t_, in_=cwT[j, hf])
            cws.append(t_)
    bwt_t = bank("b_bwt", (P, DT, D), BF16)
    nc.sync.dma_start(out=bwt_t, in_=bwtT)
    cwt_t = bank("b_cwt", (P, DT, D), BF16)
    nc.sync.dma_start(out=cwt_t, in_=cwtT)

    # ---------------- ACT queue: small params first, 2MB x broadcast last ----
    convb_t = small.tile([P, DT], F32, tag="convb")
    ll_t = small.tile([P, DT], F32, tag="ll")
    bb_t = small.tile([P, DT], F32, tag="bb")
    b2_t = small.tile([P, DT], F32, tag="b2")
    nc.scalar.dma_start(out=convb_t, in_=_vec_view(conv_b))
    nc.scalar.dma_start(out=ll_t, in_=_vec_view(log_lambda))
    nc.scalar.dma_start(out=bb_t, in_=_vec_view(bb))
    nc.scalar.dma_start(out=b2_t, in_=_vec_view(b2v))
    parity_sb = small.tile([P, 1], F32, tag="parity")
    nc.scalar.dma_start(out=parity_sb, in_=parity.partition_broadcast(P))
    cb_bc = small.tile([P, D], F32, tag="cb")
    nc.scalar.dma_start(out=cb_bc, in_=cb.partition_broadcast(P))
    xbc = bank("b_x_h", (P, XPC), BF16)
    nc.scalar.dma_start(out=xbc, in_=x_f.partition_broadcast(P))

    # lam = sigmoid(exp(log_lambda)); ln(lam); lam powers for the carry patch
    e_t = small.tile([P, DT], F32, tag="e")
    lam_t = small.tile([P, DT], F32, tag="lam")
    ll2_t = small.tile([P, DT], F32, tag="ll2")
    nc.scalar.activation(out=e_t, in_=ll_t, func=AF.Exp)
    nc.scalar.activation(out=lam_t, in_=e_t, func=AF.Sigmoid)
    nc.scalar.activation(out=ll2_t, in_=lam_t, func=AF.Ln)

    # ---------------- DVE queue: iotas, constants, onehot ----------------
    oml_t = small.tile([P, DT], F32, tag="oml")
    nc.vector.tensor_scalar(out=oml_t, in0=lam_t, scalar1=-1.0, scalar2=1.0,
                            op0=OP.mult, op1=OP.add)
    lam16 = small.tile([P, DT], BF16, tag="lam16")
    nc.vector.tensor_copy(out=lam16, in_=lam_t)
    c0b = small.tile([P, DT], F32, tag="c0b")
    nc.vector.tensor_add(out=c0b, in0=b2_t, in1=bb_t)
    nc.vector.tensor_mul(out=c0b, in0=c0b, in1=oml_t)

    iota_v = small.tile([P, 1], I32, tag="iota_v")
    nc.gpsimd.iota(iota_v, [[0, 1]], base=0, channel_multiplier=1)
    iota_vf = small.tile([P, 1], F32, tag="iota_vf")
    nc.vector.tensor_copy(out=iota_vf, in_=iota_v)
    iota_b2 = small.tile([P, 1], F32, tag="iota_b2")
    nc.vector.tensor_scalar(out=iota_b2, in0=iota_vf, scalar1=float(P),
                            scalar2=None, op0=OP.add)
    tpos_i = small.tile([P, P], I32, tag="tpos_i")
    nc.gpsimd.iota(tpos_i, [[1, P]], base=1, channel_multiplier=0)
    tpos_f = small.tile([P, P], F32, tag="tpos_f")
    nc.vector.tensor_copy(out=tpos_f, in_=tpos_i)
    irow_i = small.tile([P, P], I32, tag="irow_i")
    nc.gpsimd.iota(irow_i, [[1, P]], base=0, channel_multiplier=0)
    irow_f = small.tile([P, P], F32, tag="irow_f")
    nc.vector.tensor_copy(out=irow_f, in_=irow_i)
    ident = small.tile([P, P], F32, tag="ident")
    nc.vector.tensor_scalar(out=ident, in0=irow_f, scalar1=iota_vf,
                            scalar2=None, op0=OP.is_equal)
    ones16 = small.tile([P, P], BF16, tag="ones16")
    nc.vector.memset(ones16, 1.0)
    eps_sb = small.tile([P, 1], F32, tag="eps")
    nc.vector.memset(eps_sb, LN_EPS)

    # lam^{t+1} for the first 128 tokens (patch window), computed early
    lp8 = small.tile([P, DT, P], F32, tag="lp8")
    for dt_ in range(DT):
        nc.scalar.activation(out=lp8[:, dt_, :], in_=tpos_f, func=AF.Exp,
                             scale=ll2_t[:, dt_:dt_ + 1])

    # onehot: oh[:, k=j*2+vt, t] = (x[4t+j] == v) in bf16
    oh_t = bank("b_oh_u", (P, DT, TPC), BF16)
    xv4 = xbc.rearrange("p (q t j) -> p q t j", q=4, j=NJ)
    for j in range(NJ):
        for vt in range(VT):
            iv = iota_vf if vt == 0 else iota_b2
            k = j * VT + vt
            for q in range(4):
                nc.vector.tensor_scalar(
                    out=oh_t[:, k, q * 256:(q + 1) * 256],
                    in0=xv4[:, q, :, j], scalar1=iv, scalar2=None,
                    op0=OP.is_equal)

    # ---------------- phase A: LUT_j[v, o] = embed^T conv_w ----------------
    lut_t = bank("b_lut", (P, DT, D), BF16)
    for j in range(NJ):
        ps = [[psum(f"psA_{j}_{vt}_{oc}") for oc in range(NO)] for vt in range(VT)]
        for dt_ in range(DT):
            hf, dq = dt_ // 2, dt_ % 2
            for vt in range(VT):
                for oc in range(NO):
                    nc.tensor.matmul(
                        ps[vt][oc],
                        et_t[:, dt_, vt * P:(vt + 1) * P],
                        cws[j * 4 + hf][:, dq * D + oc * 512:dq * D + (oc + 1) * 512],
                        start=(dt_ == 0), stop=(dt_ == DT - 1))
        for vt in range(VT):
            for oc in range(NO):
                # last tap's copies on DVE so B's first chain isn't gated on
                # the ACT queue draining
                if j == NJ - 1:
                    nc.vector.tensor_copy(
                        out=lut_t[:, j * VT + vt, oc * 512:(oc + 1) * 512],
                        in_=ps[vt][oc])
                else:
                    nc.scalar.activation(
                        out=lut_t[:, j * VT + vt, oc * 512:(oc + 1) * 512],
                        in_=ps[vt][oc], func=AF.Copy)

    # ---------------- phase B: conv GEMM y^T[o, t]; squares on Pool ----------
    y_t = bank("b_y", (P, DT, TPC), BF16)
    y2_t = bank("b_y2", (P, DT, TPC), BF16)

    _sc = [0]

    def stats_chain(tc_, src_t, dst, scale):
        sl = slice(tc_ * 512, (tc_ + 1) * 512)
        _sc[0] += 1
        ps_s = psum(f"psS_{_sc[0]}")
        for ot in range(DT):
            nc.tensor.matmul(ps_s, ones16, src_t[:, ot, sl],
                             start=(ot == 0), stop=(ot == DT - 1))
        nc.scalar.activation(out=dst, in_=ps_s, func=AF.Copy, scale=scale)

    ypp_t = bank("b_ypp", (P, DT, TPC), BF16)

    def emit_b_chunk(tc_, ot):
        sl = slice(tc_ * 512, (tc_ + 1) * 512)
        psy = psum(f"psB_{tc_}_{ot}")
        for j in range(NJ):
            for vt in range(VT):
                nc.tensor.matmul(
                    psy,
                    lut_t[:, j * VT + vt, ot * P:(ot + 1) * P],
                    oh_t[:, j * VT + vt, sl],
                    start=(j == 0 and vt == 0),
                    stop=(j == NJ - 1 and vt == VT - 1))
        nc.scalar.activation(out=y_t[:, ot, sl], in_=psy, func=AF.Identity,
                             bias=convb_t[:, ot:ot + 1], scale=1.0)
        nc.gpsimd.tensor_mul(out=y2_t[:, ot, sl], in0=y_t[:, ot, sl],
                             in1=y_t[:, ot, sl])

    def emit_ln_chain(tc_, mub_c, e2_c):
        # var = E[y^2]-mu^2, r = 1/sqrt(var+eps), then normalize this chunk:
        # ypp = (y - mu) * r  (sub on Pool, mul on DVE, both [P, 512] tiles)
        sl = slice(tc_ * 512, (tc_ + 1) * 512)
        mu2 = stpool.tile([P, 512], F32, tag="mu2", name=f"mu2_{tc_}")
        nc.vector.tensor_mul(out=mu2, in0=mub_c, in1=mub_c)
        rb_c = stpool.tile([P, 512], F32, tag="rbp", name=f"rb_{tc_}")
        nc.vector.tensor_sub(out=rb_c, in0=e2_c, in1=mu2)
        nc.scalar.activation(out=rb_c, in_=rb_c, func=AF.Sqrt, bias=eps_sb)
        nc.vector.reciprocal(out=rb_c, in_=rb_c)
        for dt_ in range(DT):
            t_ = ysubpool.tile([P, 512], BF16, tag="ysub",
                               name=f"ysub_{tc_}_{dt_}")
            nc.gpsimd.tensor_sub(out=t_, in0=y_t[:, dt_, sl], in1=mub_c)
            nc.vector.tensor_mul(out=ypp_t[:, dt_, sl], in0=t_, in1=rb_c)

    # PE order: B-tc0, sum0, B-tc1[ot0], sumsq0, B-tc1[ot1..7], sum1, sumsq1
    mub0 = stpool.tile([P, 512], BF16, tag="mub", name="mub0")
    e20 = stpool.tile([P, 512], F32, tag="e2p", name="e20")
    mub1 = stpool.tile([P, 512], BF16, tag="mub", name="mub1")
    e21 = stpool.tile([P, 512], F32, tag="e2p", name="e21")
    for ot in range(DT):
        emit_b_chunk(0, ot)
    stats_chain(0, y_t, mub0, 1.0 / D)
    emit_b_chunk(1, 0)
    stats_chain(0, y2_t, e20, 1.0 / D)
    emit_ln_chain(0, mub0, e20)
    for ot in range(1, DT):
        emit_b_chunk(1, ot)
    stats_chain(1, y_t, mub1, 1.0 / D)
    stats_chain(1, y2_t, e21, 1.0 / D)
    emit_ln_chain(1, mub1, e21)

    # ------------- phase D: b-projection + scan, interleaved ----------------
    u_t = bank("b_oh_u", (P, DT, TPC), BF16)
    h_t = bank("b_x_h", (P, DT, TPC), BF16)
    fin32 = small.tile([P, DT], F32, tag="fin32")
    lam_bc = [lam16[:, k:k + 1].broadcast_to((P, TPC)) for k in range(DT)]

    def emit_scan(so):
        nc.vector.tensor_tensor_scan(
            out=h_t[:, so, :], data0=lam_bc[so], data1=u_t[:, so, :],
            initial=0.0, op0=OP.mult, op1=OP.add)
        nc.vector.tensor_copy(out=fin32[:, so:so + 1],
                              in_=h_t[:, so, TPC - 1:TPC])

    for tc_ in range(NT):
        sl = slice(tc_ * 512, (tc_ + 1) * 512)
        for ot in range(DT):
            psx = psum(f"psD_{ot}_{tc_}")
            for dt_ in range(DT):
                nc.tensor.matmul(
                    psx, bwt_t[:, dt_, ot * P:(ot + 1) * P],
                    ypp_t[:, dt_, sl],
                    start=(dt_ == 0), stop=(dt_ == DT - 1))
            nc.scalar.activation(out=u_t[:, ot, sl], in_=psx, func=AF.Identity,
                                 scale=oml_t[:, ot:ot + 1],
                                 bias=c0b[:, ot:ot + 1])
            if tc_ == 1:
                emit_scan(ot)

    # ---------------- phase F (main): c-projection for tt = 1..7 -------------
    # The final-state exchange is interleaved: after tt=1's chains, PE
    # transposes fin32 -> contiguous f32 DMA -> pair AllGather, which
    # completes while the remaining tt tiles run.
    fin_sb = small.tile([DT, P], F32, tag="fin_sb")
    carrysb = small.tile([DT, P], F32, tag="carrysb")
    for tt in range(1, DT):
        stage = stpool.tile([P, D], F32, tag="stage", name=f"stage_{tt}")
        for oc in range(NO):
            pso = psum(f"psF_{tt}_{oc}")
            for dt_ in range(DT):
                nc.tensor.matmul(
                    pso, h_t[:, dt_, tt * P:(tt + 1) * P],
                    cwt_t[:, dt_, oc * 512:(oc + 1) * 512],
                    start=(dt_ == 0), stop=(dt_ == DT - 1))
            nc.vector.scalar_tensor_tensor(
                out=stage[:, oc * 512:(oc + 1) * 512],
                in0=cb_bc[:, oc * 512:(oc + 1) * 512], scalar=1.0, in1=pso,
                op0=OP.mult, op1=OP.add)
        nc.sync.dma_start(out=out[tt * P:(tt + 1) * P, :], in_=stage)
        if tt == 1:
            ps_ft = psum("ps_ftrans")
            nc.tensor.transpose(ps_ft[0:DT, 0:P], fin32, ident)
            nc.scalar.activation(out=fin_sb, in_=ps_ft[0:DT, 0:P], func=AF.Copy)
            nc.sync.dma_start(out=fin_dram.rearrange("(r c) -> r c", r=DT),
                              in_=fin_sb)
            nc.gpsimd.collective_compute(
                "AllGather", OP.bypass,
                replica_groups=[[0, 1], [2, 3], [4, 5], [6, 7]],
                ins=[fin_dram], outs=[fin_all])
            nc.scalar.dma_start(
                out=carrysb, in_=fin_all[0].rearrange("(r c) -> r c", r=DT))

    # transpose the carry back to [p, dt] once the AllGather lands
    ps_bt = psum("ps_btrans")
    nc.tensor.transpose(ps_bt[:, 0:DT], carrysb, ident[0:DT, 0:DT])
    carrym = small.tile([P, DT], F32, tag="carrym")
    nc.vector.tensor_scalar(out=carrym, in0=ps_bt[:, 0:DT], scalar1=parity_sb,
                            scalar2=None, op0=OP.mult)
    hc_t = small.tile([P, DT, P], BF16, tag="hc")
    for dt_ in range(DT):
        corr = lppool.tile([P, P], BF16, tag="corr", name=f"corr_{dt_}")
        nc.vector.tensor_scalar(out=corr, in0=lp8[:, dt_, :],
                                scalar1=carrym[:, dt_:dt_ + 1], scalar2=None,
                                op0=OP.mult)
        nc.gpsimd.tensor_add(out=hc_t[:, dt_, :], in0=h_t[:, dt_, 0:P],
                             in1=corr)
    stage0 = stpool.tile([P, D], F32, tag="stage", name="stage_tt0")
    for oc in range(NO):
        pso = psum(f"psF0_{oc}")
        for dt_ in range(DT):
            nc.tensor.matmul(pso, hc_t[:, dt_, :],
                             cwt_t[:, dt_, oc * 512:(oc + 1) * 512],
                             start=(dt_ == 0), stop=(dt_ == DT - 1))
        nc.vector.scalar_tensor_tensor(
            out=stage0[:, oc * 512:(oc + 1) * 512],
            in0=cb_bc[:, oc * 512:(oc + 1) * 512], scalar=1.0, in1=pso,
            op0=OP.mult, op1=OP.add)
        nc.sync.dma_start(out=out[0:P, oc * 512:(oc + 1) * 512],
                          in_=stage0[:, oc * 512:(oc + 1) * 512])


_NC_CACHE = None


def _get_nc():
    global _NC_CACHE
    if _NC_CACHE is None:
        _NC_CACHE = build_nc()
    return _NC_CACHE


def _in_maps(x, embed, conv_w, conv_b, ln_g, ln_b, log_lambda, bW, bb, cW, cb):
    f = lambda a: np.ascontiguousarray(np.asarray(a, dtype=np.float32))
    bf = lambda a: np.ascontiguousarray(np.asarray(a, dtype=np.float32).astype(NPBF))
    x = np.asarray(x)
    # embed^T -> [p, dt, v]
    etT = bf(np.asarray(embed, np.float32).T.reshape(DT, P, V).transpose(1, 0, 2))
    # conv_w [o,d,k] -> [j, half, p, dq*o]
    cw = np.asarray(conv_w, np.float32).transpose(2, 1, 0)       # [k, d, o]
    cw = cw.reshape(NJ, DT, P, D).transpose(0, 2, 1, 3)          # [j, p, dt, o]
    cwT = bf(cw.reshape(NJ, P, 4, 2, D).transpose(0, 2, 1, 3, 4)
             .reshape(NJ, 4, P, 2 * D))
    # weight prep (O(D^2), init-time): fold ln gamma into bW; b2 = bW @ ln_b
    bW32 = np.asarray(bW, np.float32)
    bWg = bW32 * np.asarray(ln_g, np.float32)[None, :]
    b2 = bW32 @ np.asarray(ln_b, np.float32)
    bwtT = bf(bWg.T.reshape(DT, P, D).transpose(1, 0, 2))
    cwtT = bf(np.asarray(cW, np.float32).T.reshape(DT, P, D).transpose(1, 0, 2))
    shared = dict(etT=etT, cwT=cwT, bwtT=bwtT, cwtT=cwtT,
                  conv_b=f(conv_b), log_lambda=f(log_lambda),
                  b2v=f(b2), bb=f(bb), cb=f(cb))
    maps = []
    for c in range(NCORES):
        b, h = c // 2, c % 2
        xf = np.ascontiguousarray(
            x[b, h * XPC:(h + 1) * XPC].astype(np.float32).astype(NPBF))
        maps.append(dict(x_f=xf, parity=np.array([float(h)], np.float32),
                         **shared))
    return maps


def _unshard(results):
    out = np.empty((B, TC, D), np.float32)
    for c in range(NCORES):
        b, h = c // 2, c % 2
        out[b, h * TPC:(h + 1) * TPC, :] = results[c]["out"]
    return out


def run(trace=False, **inputs):
    from concourse.bass_utils import run_bass_kernel_spmd
    nc = _get_nc()
    maps = _in_maps(**inputs)
    res = run_bass_kernel_spmd(nc, maps, list(range(NCORES)), trace=trace)
    return _unshard(res.results), res


def kernel(**inputs):
    out, _ = run(trace=False, **inputs)
    return out
